# revision 5
# baseline (speedup 1.0000x reference)
"""MultiHeadSelfAttentionWithRoPE on 8 TRN2 NeuronCores.

Sharding: queries are sharded round-robin (core c owns global rows
{c, c+8, ...}) so causal attention work is balanced and the program is
pure SPMD. K/V for seq chunks [0, 512*M_LOCAL) are computed replicated
on every core; K/V for the remaining chunks are computed sharded (each
core projects W = 64*(8-M_LOCAL) positions) and exchanged with a single
AllGather whose ~100us cold-start overlaps the local projections and
the first half of attention (which only needs the local kv blocks).

Layouts (transposed so no on-chip transposes are needed):
  QT/KT [128, eo(2), quad(3), n] bf16 — head-dim permutation folded into
        wq/wk column order on host.
  ST    = KT.T @ QT per 128-wide kv-block into PSUM as 2 accumulating
        K=32 matmuls (evens+odds) x 4 heads in distinct 32-row PE groups.
  V     [4096, 768(+ones col per head)] bf16; the ones column makes the
        softmax denominator fall out of the PV matmul.
  OT    [65, 512] per head accumulated in PSUM over kv-blocks; row 64 is
        the denominator.
  out   = OT.T @ woT accumulated over 6 head-dim slabs, fp32.

Softmax skips max-subtraction (valid scores bounded |s| < ~8). The
q-slice for kv-block k is [16k, 512); only the first 16 cols are
causally partial (per-core [128,16] 0/1 mask).
"""

import numpy as np

D = 768
S = 4096
H = 12
HD = 64
HD2 = 32
NCORES = 8
QPC = S // NCORES          # 512 query rows per core
NKV = S // 128             # 32 kv blocks
NDS = D // 128             # 6 d-slices
SCALE = float(1.0 / np.sqrt(HD))

M_LOCAL = 4                # seq chunks computed replicated locally
SLOC = 512 * M_LOCAL       # locally projected kv positions
W = 64 * (8 - M_LOCAL)     # per-core gathered-projection width
NSB = W // 128             # 128-wide sub-blocks in the gather slice
KV0 = 4 * M_LOCAL          # first gathered kv block

_CACHE = {}
last_exec_time_ns = None
last_results = None


def _head_perm():
    """Column permutation for wq/wk: slab s = 3*eo + quad, partition p =
    32*a + i  ->  original dim e = 64*(4*quad + a) + 2*i + eo."""
    perm = np.zeros(D, dtype=np.int64)
    for s in range(6):
        eo, quad = divmod(s, 3)
        for p in range(128):
            a, i = divmod(p, 32)
            perm[128 * s + p] = 64 * (4 * quad + a) + 2 * i + eo
    return perm


def _build_program():
    import concourse.mybir as mybir
    import concourse.tile as tile
    from concourse import bacc
    from contextlib import ExitStack

    dt = mybir.dt
    bf = dt.bfloat16
    f32 = dt.float32
    nc = bacc.Bacc("TRN2", target_bir_lowering=False, debug=False,
                   num_devices=NCORES)

    def din(name, shape, dtype):
        return nc.dram_tensor(name, shape, dtype, kind="ExternalInput").ap()

    xT_d = din("xT", [D, SLOC], bf)     # local-chunk cols of x.T
    xg_d = din("xg", [D, W], bf)        # this core's gather-slice cols
    xq_d = din("xq", [D, QPC], bf)
    wqT_d = din("wqT", [D, D], bf)      # permuted cols
    wkT_d = din("wkT", [D, D], bf)      # permuted cols
    wvT_d = din("wvT", [D, D], bf)      # natural
    woT_d = din("woT", [D, D], bf)      # wo.T natural
    cosK_d = din("cosK", [128, SLOC], bf)
    sinK_d = din("sinK", [128, SLOC], bf)
    cosG_d = din("cosG", [128, W], bf)
    sinG_d = din("sinG", [128, W], bf)
    cosQ_d = din("cosQ", [128, QPC], bf)
    sinQ_d = din("sinQ", [128, QPC], bf)
    mask_d = din("mask", [128, 16], bf)
    out_d = nc.dram_tensor("out", [QPC, D], f32, kind="ExternalOutput").ap()

    # collective bounce: [K slabs 6*W | V (W/128)*12*65] per core
    # (V payload includes the ones column so the gather-back DMA into
    # VSB is fully contiguous)
    CKV = 6 * W + NSB * H * (HD + 1)
    cin_d = nc.dram_tensor("cin", [128, CKV], bf, kind="Internal").ap()
    gout_d = nc.dram_tensor("gout", [NCORES, 128, CKV], bf, kind="Internal",
                            addr_space="Shared").ap()
    rg = [list(range(NCORES))]

    with tile.TileContext(nc) as tc, ExitStack() as ctx:
        # ---- long-lived SBUF ----
        P_LL = ctx.enter_context(tc.tile_pool(name="ll", bufs=1))
        KT = P_LL.tile([128, 2, 3, S], bf)                  # 6.3 MB
        QT = P_LL.tile([128, 2, 3, QPC], bf)                # 0.8 MB
        VSB = P_LL.tile([128, NKV, H, HD + 1], bf)          # 6.4 MB
        OTSB = P_LL.tile([128, 6, QPC], bf)                 # 0.8 MB
        KG = P_LL.tile([128, 2, 3, W], bf)
        VG = P_LL.tile([128, NSB, H, HD + 1], bf)
        msk = P_LL.tile([128, 16], bf)
        nc.sync.dma_start(msk[:], mask_d)
        nc.gpsimd.memset(VSB[:, 0:KV0, :, HD:HD + 1], 1.0)
        nc.gpsimd.memset(VG[:, :, :, HD:HD + 1], 1.0)

        # ---- transient SBUF pools ----
        P_W = ctx.enter_context(tc.tile_pool(name="wt", bufs=2))
        P_X = ctx.enter_context(tc.tile_pool(name="xs", bufs=12))
        P_RT = ctx.enter_context(tc.tile_pool(name="rt", bufs=3))
        P_CS = ctx.enter_context(tc.tile_pool(name="cs", bufs=3))
        P_P = ctx.enter_context(tc.tile_pool(name="pp", bufs=4))
        P_N = ctx.enter_context(tc.tile_pool(name="nrm", bufs=2))
        P_N1 = ctx.enter_context(tc.tile_pool(name="nrm1", bufs=1))
        P_O = ctx.enter_context(tc.tile_pool(name="outs", bufs=2))

        def load_w(dram):
            w = P_W.tile([128, NDS, D], bf, tag="w")
            for ds in range(NDS):
                nc.scalar.dma_start(w[:, ds, :],
                                    dram[128 * ds:128 * (ds + 1), :])
            return w

        def rope2(de, do, src_e_ps, src_o_ps, cos_t, sin_t, n):
            se = P_RT.tile([128, 512], bf, tag="se")
            so = P_RT.tile([128, 512], bf, tag="so")
            nc.vector.tensor_copy(se[:, 0:n], src_e_ps)
            nc.vector.tensor_copy(so[:, 0:n], src_o_ps)
            t1 = P_RT.tile([128, 512], bf, tag="t1")
            t2 = P_RT.tile([128, 512], bf, tag="t2")
            nc.vector.tensor_mul(t1[:, 0:n], se[:, 0:n], cos_t)
            nc.vector.tensor_mul(t2[:, 0:n], so[:, 0:n], sin_t)
            nc.vector.tensor_sub(de, t1[:, 0:n], t2[:, 0:n])
            t3 = P_RT.tile([128, 512], bf, tag="t1")
            t4 = P_RT.tile([128, 512], bf, tag="t2")
            nc.vector.tensor_mul(t3[:, 0:n], se[:, 0:n], sin_t)
            nc.vector.tensor_mul(t4[:, 0:n], so[:, 0:n], cos_t)
            nc.vector.tensor_add(do, t3[:, 0:n], t4[:, 0:n])

        # ============ phases A+B: projections ======================
        with tc.tile_pool(name="pps", bufs=2, space="PSUM") as P_PS:

            def qk_proj(w_sb, xtiles, dst, c0, n, cos_t, sin_t):
                for quad in range(3):
                    pe = P_PS.tile([128, 512], f32, tag="kpsE")
                    po = P_PS.tile([128, 512], f32, tag="kpsO")
                    for s, ps in ((quad, pe), (3 + quad, po)):
                        for ds in range(NDS):
                            nc.tensor.matmul(
                                ps[:, 0:n], w_sb[:, ds, 128 * s:128 * (s + 1)],
                                xtiles[ds][:, 0:n], start=(ds == 0),
                                stop=(ds == NDS - 1))
                    rope2(dst[:, 0, quad, c0:c0 + n], dst[:, 1, quad, c0:c0 + n],
                          pe[:, 0:n], po[:, 0:n], cos_t, sin_t, n)

            def v_proj_one(xtiles, wv_sb, xoff, dst_fn):
                """V proj for one 128-wide seq sub-block starting at xoff."""
                for nh in range(2):
                    vps = P_PS.tile([128, 384], f32, tag="vps")
                    for ds in range(NDS):
                        nc.tensor.matmul(
                            vps[:],
                            xtiles[ds][:, xoff:xoff + 128],
                            wv_sb[:, ds, 384 * nh:384 * (nh + 1)],
                            start=(ds == 0), stop=(ds == NDS - 1))
                    nc.scalar.copy(
                        dst_fn(nh),
                        vps[:].rearrange("p (h d) -> p h d", h=6))

            # ---- gather-slice K/V projection first (feeds collective) ----
            wk_sb = load_w(wkT_d)
            wv_sb = load_w(wvT_d)
            xgs = []
            for ds in range(NDS):
                xg = P_X.tile([128, 512], bf, tag="xt", name=f"xg_{ds}")
                nc.sync.dma_start(xg[:, 0:W], xg_d[128 * ds:128 * (ds + 1), :])
                xgs.append(xg)
            cg = P_CS.tile([128, 512], bf, tag="ck")
            sg = P_CS.tile([128, 512], bf, tag="sk")
            nc.sync.dma_start(cg[:, 0:W], cosG_d)
            nc.sync.dma_start(sg[:, 0:W], sinG_d)

            qk_proj(wk_sb, xgs, KG, 0, W, cg[:, 0:W], sg[:, 0:W])
            for sbi in range(NSB):
                v_proj_one(xgs, wv_sb, 128 * sbi,
                           lambda nh, sbi=sbi:
                           VG[:, sbi, 6 * nh:6 * (nh + 1), 0:HD])

            # stage + fire the collective
            nc.gpsimd.dma_start(
                cin_d[:, 0:6 * W],
                KG[:].rearrange("p a b w -> p (a b w)"))
            nc.gpsimd.dma_start(
                cin_d[:, 6 * W:],
                VG[:].rearrange("p b h d -> p (b h d)"))
            nc.gpsimd.collective_compute(
                "AllGather", mybir.AluOpType.bypass, replica_groups=rg,
                ins=[cin_d], outs=[gout_d])

            # ---- local replicated chunks 0..M_LOCAL-1 ----
            def load_chunk(ch):
                c0 = 512 * ch
                xts = []
                for ds in range(NDS):
                    xt = P_X.tile([128, 512], bf, tag="xt",
                                  name=f"xt{ch}_{ds}")
                    nc.sync.dma_start(
                        xt[:], xT_d[128 * ds:128 * (ds + 1), c0:c0 + 512])
                    xts.append(xt)
                ck = P_CS.tile([128, 512], bf, tag="ck", name=f"ck{ch}")
                sk = P_CS.tile([128, 512], bf, tag="sk", name=f"sk{ch}")
                nc.sync.dma_start(ck[:], cosK_d[:, c0:c0 + 512])
                nc.sync.dma_start(sk[:], sinK_d[:, c0:c0 + 512])
                return xts, ck, sk

            nxt = load_chunk(0)
            for ch in range(M_LOCAL):
                c0 = 512 * ch
                xts, ck, sk = nxt
                if ch + 1 < M_LOCAL:
                    nxt = load_chunk(ch + 1)

                qk_proj(wk_sb, xts, KT, c0, 512, ck[:], sk[:])
                for sb in range(4):
                    v_proj_one(xts, wv_sb, 128 * sb,
                               lambda nh, sb=sb, ch=ch:
                               VSB[:, 4 * ch + sb,
                                   6 * nh:6 * (nh + 1), 0:HD])

            # ---- Q projection ----
            wq_sb = load_w(wqT_d)
            cq = P_CS.tile([128, QPC], bf, tag="ck")
            sq = P_CS.tile([128, QPC], bf, tag="sk")
            nc.sync.dma_start(cq[:], cosQ_d)
            nc.sync.dma_start(sq[:], sinQ_d)
            xqs = []
            for ds in range(NDS):
                xq = P_X.tile([128, QPC], bf, tag="xt")
                nc.sync.dma_start(xq[:], xq_d[128 * ds:128 * (ds + 1), :])
                xqs.append(xq)
            qk_proj(wq_sb, xqs, QT, 0, QPC, cq[:], sq[:])

        # Load wo early so its DMA overlaps attention (before the
        # gather-back DMAs so it isn't gated on the collective).
        wo_sb = load_w(woT_d)

        # ---- unpack the gathered K/V into KT / VSB tails ----
        for r in range(NCORES):
            off = SLOC + W * r
            nc.gpsimd.dma_start(
                VSB[:, KV0 + NSB * r:KV0 + NSB * (r + 1), :, :]
                .rearrange("p b h d -> p (b h d)"),
                gout_d[r, :, 6 * W:])
            nc.gpsimd.dma_start(
                KT[:, :, :, off:off + W].rearrange("p a b w -> p (a b) w"),
                gout_d[r, :, 0:6 * W].rearrange("p (s w) -> p s w", w=W))

        nrmA_d = nc.dram_tensor("nrm_den", [3, 4 * QPC], f32,
                                kind="Internal").ap()
        nrmB_d = nc.dram_tensor("nrm_rcp", [3, 4 * QPC], bf,
                                kind="Internal").ap()

        # ============ phase C: attention ===========================
        with tc.tile_pool(name="st", bufs=1, space="PSUM") as P_ST, \
             tc.tile_pool(name="ot", bufs=1, space="PSUM") as P_OT:
            for g in range(3):                  # head quads
                otb = P_OT.tile([65, 4, QPC], f32, tag="ot")
                prev = None

                def pv_flush(g=g, otb=otb):
                    nonlocal prev
                    if prev is None:
                        return
                    pk, pp = prev
                    pq0 = 16 * pk
                    for a in range(4):
                        nc.tensor.matmul(
                            otb[:, a, pq0:QPC], VSB[:, pk, 4 * g + a, :],
                            pp[:, a, :], start=(pk == 0),
                            stop=(pk == NKV - 1))
                    prev = None

                for k in range(NKV):            # kv blocks
                    q0 = 16 * k
                    n = QPC - q0
                    halves = []
                    for hb in range(2):         # half: heads {2hb, 2hb+1}
                        stb = P_ST.tile([128, 2, 512], f32, tag=f"st{hb}")
                        for eo in range(2):
                            for aa in range(2):
                                a = 2 * hb + aa
                                tp = (96, 0) if a == 3 else None
                                nc.tensor.matmul(
                                    stb[:, aa, 0:n],
                                    KT[32 * a:32 * (a + 1), eo, g,
                                       128 * k:128 * (k + 1)],
                                    QT[32 * a:32 * (a + 1), eo, g, q0:QPC],
                                    start=(eo == 0), stop=(eo == 1),
                                    tile_position=tp)
                        halves.append(stb)
                    pv_flush()
                    p = P_P.tile([128, 4, n], bf, tag="p")
                    for hb in range(2):
                        nc.scalar.activation(
                            p[:, 2 * hb:2 * hb + 2, :], halves[hb][:, :, 0:n],
                            mybir.ActivationFunctionType.Exp, scale=SCALE)
                    nc.vector.tensor_mul(
                        p[:, :, 0:16], p[:, :, 0:16],
                        msk[:, None, :].broadcast_to((128, 4, 16)))
                    prev = (k, p)
                pv_flush()

                # normalize: reciprocals + unnormalized copies on DVE; the
                # broadcast DMA bounce + in-place multiply overlap the next
                # quad's attention.
                r1 = P_N.tile([1, 4 * QPC], bf, tag="r1")
                with nc.allow_low_precision(reason="bf16 softmax denom"):
                    for a in range(4):
                        nc.vector.reciprocal(r1[:, QPC * a:QPC * (a + 1)],
                                             otb[64:65, a, :])
                for half in range(2):
                    nc.vector.tensor_copy(
                        OTSB[64 * half:64 * half + 64, 2 * g:2 * g + 2, :],
                        otb[0:64, half::2, :])
                nc.sync.dma_start(nrmB_d[g:g + 1, :], r1[:])
                rb = P_N.tile([128, 4 * QPC], bf, tag="rb")
                nc.sync.dma_start(rb[:],
                                  nrmB_d[g:g + 1, :].to_broadcast((128, 4 * QPC)))
                rb4 = rb[:].rearrange("p (a q) -> p a q", a=4)
                for a in range(4):
                    h = 4 * g + a
                    pb = 64 * (h % 2)
                    dst = OTSB[pb:pb + 64, h // 2, :]
                    nc.vector.tensor_mul(dst, dst, rb4[pb:pb + 64, a, :])

        # ============ phase D: output projection ===================
        with tc.tile_pool(name="pd", bufs=1, space="PSUM") as P_PD:
            for j in range(4):                  # q sub-tiles of 128
                pss = []
                for nh in range(2):
                    ps = P_PD.tile([128, 384], f32, tag=f"ops{nh}")
                    for s in range(NDS):
                        nc.tensor.matmul(
                            ps[:], OTSB[:, s, 128 * j:128 * (j + 1)],
                            wo_sb[:, s, 384 * nh:384 * (nh + 1)],
                            start=(s == 0), stop=(s == NDS - 1))
                    pss.append(ps)
                ob = P_O.tile([128, D], f32, tag="ob")
                nc.scalar.copy(ob[:, 0:384], pss[0][:])
                nc.scalar.copy(ob[:, 384:768], pss[1][:])
                nc.sync.dma_start(out_d[128 * j:128 * (j + 1), :], ob[:])

    nc.compile()
    return nc


def _prep_inputs(x, wq, wk, wv, wo, token_positions):
    import ml_dtypes
    bf16 = ml_dtypes.bfloat16

    x2 = np.ascontiguousarray(x[0], dtype=np.float32)          # [S, D]
    xT = np.ascontiguousarray(x2.T).astype(bf16)               # [D, S]
    perm = _head_perm()
    wqT = np.ascontiguousarray(wq[perm, :].T).astype(bf16)
    wkT = np.ascontiguousarray(wk[perm, :].T).astype(bf16)
    wvT = np.ascontiguousarray(wv.T).astype(bf16)
    woT = np.ascontiguousarray(wo.T).astype(bf16)

    pos = np.asarray(token_positions[0], dtype=np.int64)       # [S]
    kk = np.arange(HD2, dtype=np.float32)
    inv = (10000.0 ** (-2.0 * kk / HD)).astype(np.float32)
    ang = pos[:, None].astype(np.float32) * inv[None, :]       # [S, 32]
    cosf = np.cos(ang, dtype=np.float32)
    sinf = np.sin(ang, dtype=np.float32)
    cosK = np.ascontiguousarray(np.tile(cosf[:SLOC].T, (4, 1))).astype(bf16)
    sinK = np.ascontiguousarray(np.tile(sinf[:SLOC].T, (4, 1))).astype(bf16)

    xTloc = np.ascontiguousarray(xT[:, :SLOC])

    per_core = []
    for c in range(NCORES):
        lo = SLOC + W * c
        xg = np.ascontiguousarray(xT[:, lo:lo + W])
        cosG = np.ascontiguousarray(
            np.tile(cosf[lo:lo + W].T, (4, 1))).astype(bf16)
        sinG = np.ascontiguousarray(
            np.tile(sinf[lo:lo + W].T, (4, 1))).astype(bf16)
        xq = np.ascontiguousarray(xT[:, c::NCORES])            # [D, 512]
        cosQ = np.ascontiguousarray(
            np.tile(cosf[c::NCORES].T, (4, 1))).astype(bf16)
        sinQ = np.ascontiguousarray(
            np.tile(sinf[c::NCORES].T, (4, 1))).astype(bf16)
        kl = np.arange(128)[:, None]
        jj = np.arange(16)[None, :]
        mask = (kl <= 8 * jj + c).astype(np.float32).astype(bf16)
        per_core.append({
            "xT": xTloc, "xg": xg, "xq": xq,
            "wqT": wqT, "wkT": wkT, "wvT": wvT, "woT": woT,
            "cosK": cosK, "sinK": sinK, "cosG": cosG, "sinG": sinG,
            "cosQ": cosQ, "sinQ": sinQ, "mask": mask,
        })
    return per_core


def kernel(x, wq, wk, wv, wo, token_positions):
    global last_exec_time_ns, last_results
    import os
    from concourse import bass_utils

    key = "v2"
    if key not in _CACHE:
        _CACHE[key] = _build_program()
    nc = _CACHE[key]

    in_maps = _prep_inputs(np.asarray(x), np.asarray(wq), np.asarray(wk),
                           np.asarray(wv), np.asarray(wo),
                           np.asarray(token_positions))

    kw = {}
    if os.environ.get("BASS_KERNEL_TRACE", "0") == "1":
        kw = dict(trace=True,
                  trace_cores=[int(t) for t in os.environ.get(
                      "BASS_KERNEL_TRACE_CORES", "0").split(",")])
        td = os.environ.get("BASS_KERNEL_TMPDIR")
        if td:
            kw["tmpdir"] = td
    res = bass_utils.run_bass_kernel_spmd(nc, in_maps,
                                          core_ids=list(range(NCORES)), **kw)
    last_exec_time_ns = res.exec_time_ns
    last_results = res

    out = np.empty((S, D), dtype=np.float32)
    for c in range(NCORES):
        out[c::NCORES, :] = res.results[c]["out"]
    return out[None, :, :]


# revision 6
# speedup vs baseline: 1.0269x; 1.0269x over previous
"""MultiHeadSelfAttentionWithRoPE on 8 TRN2 NeuronCores.

Sharding: queries are sharded round-robin (core c owns global rows
{c, c+8, ...}) so causal attention work is balanced and the program is
pure SPMD. K/V for seq chunks [0, 512*M_LOCAL) are computed replicated
on every core; K/V for the remaining chunks are computed sharded (each
core projects W = 64*(8-M_LOCAL) positions) and exchanged with a single
AllGather whose ~100us cold-start overlaps the local projections and
the first half of attention (which only needs the local kv blocks).

Layouts (transposed so no on-chip transposes are needed):
  QT/KT [128, eo(2), quad(3), n] bf16 — head-dim permutation folded into
        wq/wk column order on host.
  ST    = KT.T @ QT per 128-wide kv-block into PSUM as 2 accumulating
        K=32 matmuls (evens+odds) x 4 heads in distinct 32-row PE groups.
  V     [4096, 768(+ones col per head)] bf16; the ones column makes the
        softmax denominator fall out of the PV matmul.
  OT    [65, 512] per head accumulated in PSUM over kv-blocks; row 64 is
        the denominator.
  out   = OT.T @ woT accumulated over 6 head-dim slabs, fp32.

Softmax skips max-subtraction (valid scores bounded |s| < ~8). The
q-slice for kv-block k is [16k, 512); only the first 16 cols are
causally partial (per-core [128,16] 0/1 mask).
"""

import numpy as np

D = 768
S = 4096
H = 12
HD = 64
HD2 = 32
NCORES = 8
QPC = S // NCORES          # 512 query rows per core
NKV = S // 128             # 32 kv blocks
NDS = D // 128             # 6 d-slices
SCALE = float(1.0 / np.sqrt(HD))

M_LOCAL = 4                # seq chunks computed replicated locally
SLOC = 512 * M_LOCAL       # locally projected kv positions
W = 64 * (8 - M_LOCAL)     # per-core gathered-projection width
NSB = W // 128             # 128-wide sub-blocks in the gather slice
KV0 = 4 * M_LOCAL          # first gathered kv block

_CACHE = {}
last_exec_time_ns = None
last_results = None


def _head_perm():
    """Column permutation for wq/wk: slab s = 3*eo + quad, partition p =
    32*a + i  ->  original dim e = 64*(4*quad + a) + 2*i + eo."""
    perm = np.zeros(D, dtype=np.int64)
    for s in range(6):
        eo, quad = divmod(s, 3)
        for p in range(128):
            a, i = divmod(p, 32)
            perm[128 * s + p] = 64 * (4 * quad + a) + 2 * i + eo
    return perm


def _build_program():
    import concourse.mybir as mybir
    import concourse.tile as tile
    from concourse import bacc
    from contextlib import ExitStack

    dt = mybir.dt
    bf = dt.bfloat16
    f32 = dt.float32
    nc = bacc.Bacc("TRN2", target_bir_lowering=False, debug=False,
                   num_devices=NCORES)

    def din(name, shape, dtype):
        return nc.dram_tensor(name, shape, dtype, kind="ExternalInput").ap()

    xT_d = din("xT", [D, SLOC], bf)     # local-chunk cols of x.T
    xg_d = din("xg", [D, W], bf)        # this core's gather-slice cols
    xq_d = din("xq", [D, QPC], bf)
    wqT_d = din("wqT", [D, D], bf)      # permuted cols
    wkT_d = din("wkT", [D, D], bf)      # permuted cols
    wvT_d = din("wvT", [D, D], bf)      # natural
    woT_d = din("woT", [D, D], bf)      # wo.T natural
    cosK_d = din("cosK", [128, SLOC], bf)
    sinK_d = din("sinK", [128, SLOC], bf)
    cosG_d = din("cosG", [128, W], bf)
    sinG_d = din("sinG", [128, W], bf)
    cosQ_d = din("cosQ", [128, QPC], bf)
    sinQ_d = din("sinQ", [128, QPC], bf)
    mask_d = din("mask", [128, 16], bf)
    out_d = nc.dram_tensor("out", [QPC, D], f32, kind="ExternalOutput").ap()

    # collective bounce: [K slabs 6*W | V (W/128)*12*65] per core
    # (V payload includes the ones column so the gather-back DMA into
    # VSB is fully contiguous)
    CKV = 6 * W + NSB * H * (HD + 1)
    cin_d = nc.dram_tensor("cin", [128, CKV], bf, kind="Internal").ap()
    gout_d = nc.dram_tensor("gout", [NCORES, 128, CKV], bf, kind="Internal",
                            addr_space="Shared").ap()
    rg = [list(range(NCORES))]

    with tile.TileContext(nc) as tc, ExitStack() as ctx:
        # ---- long-lived SBUF ----
        P_LL = ctx.enter_context(tc.tile_pool(name="ll", bufs=1))
        KT = P_LL.tile([128, 2, 3, S], bf)                  # 6.3 MB
        QT = P_LL.tile([128, 2, 3, QPC], bf)                # 0.8 MB
        VSB = P_LL.tile([128, NKV, H, HD + 1], bf)          # 6.4 MB
        OTSB = P_LL.tile([128, 6, QPC], bf)                 # 0.8 MB
        KG = P_LL.tile([128, 2, 3, W], bf)
        VG = P_LL.tile([128, NSB, H, HD + 1], bf)
        msk = P_LL.tile([128, 16], bf)
        nc.sync.dma_start(msk[:], mask_d)
        nc.gpsimd.memset(VSB[:, 0:KV0, :, HD:HD + 1], 1.0)
        nc.gpsimd.memset(VG[:, :, :, HD:HD + 1], 1.0)

        # ---- transient SBUF pools ----
        P_W = ctx.enter_context(tc.tile_pool(name="wt", bufs=2))
        P_X = ctx.enter_context(tc.tile_pool(name="xs", bufs=12))
        P_RT = ctx.enter_context(tc.tile_pool(name="rt", bufs=3))
        P_CS = ctx.enter_context(tc.tile_pool(name="cs", bufs=3))
        P_P = ctx.enter_context(tc.tile_pool(name="pp", bufs=3))
        P_N = ctx.enter_context(tc.tile_pool(name="nrm", bufs=2))
        P_N1 = ctx.enter_context(tc.tile_pool(name="nrm1", bufs=1))
        P_O = ctx.enter_context(tc.tile_pool(name="outs", bufs=2))

        def load_w(dram):
            w = P_W.tile([128, NDS, D], bf, tag="w")
            for ds in range(NDS):
                nc.scalar.dma_start(w[:, ds, :],
                                    dram[128 * ds:128 * (ds + 1), :])
            return w

        def rope2(de, do, src_e_ps, src_o_ps, cos_t, sin_t, n):
            se = P_RT.tile([128, 512], bf, tag="se")
            so = P_RT.tile([128, 512], bf, tag="so")
            nc.vector.tensor_copy(se[:, 0:n], src_e_ps)
            nc.vector.tensor_copy(so[:, 0:n], src_o_ps)
            t1 = P_RT.tile([128, 512], bf, tag="t1")
            t2 = P_RT.tile([128, 512], bf, tag="t2")
            nc.vector.tensor_mul(t1[:, 0:n], se[:, 0:n], cos_t)
            nc.vector.tensor_mul(t2[:, 0:n], so[:, 0:n], sin_t)
            nc.vector.tensor_sub(de, t1[:, 0:n], t2[:, 0:n])
            t3 = P_RT.tile([128, 512], bf, tag="t1")
            t4 = P_RT.tile([128, 512], bf, tag="t2")
            nc.vector.tensor_mul(t3[:, 0:n], se[:, 0:n], sin_t)
            nc.vector.tensor_mul(t4[:, 0:n], so[:, 0:n], cos_t)
            nc.vector.tensor_add(do, t3[:, 0:n], t4[:, 0:n])

        # ============ phases A+B: projections ======================
        with tc.tile_pool(name="pps", bufs=2, space="PSUM") as P_PS:

            def qk_proj(w_sb, xtiles, dst, c0, n, cos_t, sin_t):
                for quad in range(3):
                    pe = P_PS.tile([128, 512], f32, tag="kpsE")
                    po = P_PS.tile([128, 512], f32, tag="kpsO")
                    for s, ps in ((quad, pe), (3 + quad, po)):
                        for ds in range(NDS):
                            nc.tensor.matmul(
                                ps[:, 0:n], w_sb[:, ds, 128 * s:128 * (s + 1)],
                                xtiles[ds][:, 0:n], start=(ds == 0),
                                stop=(ds == NDS - 1))
                    rope2(dst[:, 0, quad, c0:c0 + n], dst[:, 1, quad, c0:c0 + n],
                          pe[:, 0:n], po[:, 0:n], cos_t, sin_t, n)

            def v_proj_one(xtiles, wv_sb, xoff, dst_fn):
                """V proj for one 128-wide seq sub-block starting at xoff."""
                for nh in range(2):
                    vps = P_PS.tile([128, 384], f32, tag="vps")
                    for ds in range(NDS):
                        nc.tensor.matmul(
                            vps[:],
                            xtiles[ds][:, xoff:xoff + 128],
                            wv_sb[:, ds, 384 * nh:384 * (nh + 1)],
                            start=(ds == 0), stop=(ds == NDS - 1))
                    nc.scalar.copy(
                        dst_fn(nh),
                        vps[:].rearrange("p (h d) -> p h d", h=6))

            # ---- gather-slice K/V projection first (feeds collective) ----
            wk_sb = load_w(wkT_d)
            wv_sb = load_w(wvT_d)
            xgs = []
            for ds in range(NDS):
                xg = P_X.tile([128, 512], bf, tag="xt", name=f"xg_{ds}")
                nc.sync.dma_start(xg[:, 0:W], xg_d[128 * ds:128 * (ds + 1), :])
                xgs.append(xg)
            cg = P_CS.tile([128, 512], bf, tag="ck")
            sg = P_CS.tile([128, 512], bf, tag="sk")
            nc.sync.dma_start(cg[:, 0:W], cosG_d)
            nc.sync.dma_start(sg[:, 0:W], sinG_d)

            qk_proj(wk_sb, xgs, KG, 0, W, cg[:, 0:W], sg[:, 0:W])
            for sbi in range(NSB):
                v_proj_one(xgs, wv_sb, 128 * sbi,
                           lambda nh, sbi=sbi:
                           VG[:, sbi, 6 * nh:6 * (nh + 1), 0:HD])

            # stage + fire the collective
            nc.gpsimd.dma_start(
                cin_d[:, 0:6 * W],
                KG[:].rearrange("p a b w -> p (a b w)"))
            nc.gpsimd.dma_start(
                cin_d[:, 6 * W:],
                VG[:].rearrange("p b h d -> p (b h d)"))
            nc.gpsimd.collective_compute(
                "AllGather", mybir.AluOpType.bypass, replica_groups=rg,
                ins=[cin_d], outs=[gout_d])

            # ---- local replicated chunks 0..M_LOCAL-1 ----
            def load_chunk(ch):
                c0 = 512 * ch
                xts = []
                for ds in range(NDS):
                    xt = P_X.tile([128, 512], bf, tag="xt",
                                  name=f"xt{ch}_{ds}")
                    nc.sync.dma_start(
                        xt[:], xT_d[128 * ds:128 * (ds + 1), c0:c0 + 512])
                    xts.append(xt)
                ck = P_CS.tile([128, 512], bf, tag="ck", name=f"ck{ch}")
                sk = P_CS.tile([128, 512], bf, tag="sk", name=f"sk{ch}")
                nc.sync.dma_start(ck[:], cosK_d[:, c0:c0 + 512])
                nc.sync.dma_start(sk[:], sinK_d[:, c0:c0 + 512])
                return xts, ck, sk

            nxt = load_chunk(0)
            for ch in range(M_LOCAL):
                c0 = 512 * ch
                xts, ck, sk = nxt
                if ch + 1 < M_LOCAL:
                    nxt = load_chunk(ch + 1)

                qk_proj(wk_sb, xts, KT, c0, 512, ck[:], sk[:])
                for sb in range(4):
                    v_proj_one(xts, wv_sb, 128 * sb,
                               lambda nh, sb=sb, ch=ch:
                               VSB[:, 4 * ch + sb,
                                   6 * nh:6 * (nh + 1), 0:HD])

            # ---- Q projection ----
            wq_sb = load_w(wqT_d)
            cq = P_CS.tile([128, QPC], bf, tag="ck")
            sq = P_CS.tile([128, QPC], bf, tag="sk")
            nc.sync.dma_start(cq[:], cosQ_d)
            nc.sync.dma_start(sq[:], sinQ_d)
            xqs = []
            for ds in range(NDS):
                xq = P_X.tile([128, QPC], bf, tag="xt")
                nc.sync.dma_start(xq[:], xq_d[128 * ds:128 * (ds + 1), :])
                xqs.append(xq)
            qk_proj(wq_sb, xqs, QT, 0, QPC, cq[:], sq[:])

        # Load wo early so its DMA overlaps attention (before the
        # gather-back DMAs so it isn't gated on the collective).
        wo_sb = load_w(woT_d)

        # ---- unpack the gathered K/V into KT / VSB tails ----
        for r in range(NCORES):
            off = SLOC + W * r
            nc.gpsimd.dma_start(
                VSB[:, KV0 + NSB * r:KV0 + NSB * (r + 1), :, :]
                .rearrange("p b h d -> p (b h d)"),
                gout_d[r, :, 6 * W:])
            nc.gpsimd.dma_start(
                KT[:, :, :, off:off + W].rearrange("p a b w -> p (a b) w"),
                gout_d[r, :, 0:6 * W].rearrange("p (s w) -> p s w", w=W))

        nrmA_d = nc.dram_tensor("nrm_den", [3, 4 * QPC], f32,
                                kind="Internal").ap()
        nrmB_d = nc.dram_tensor("nrm_rcp", [3, 4 * QPC], bf,
                                kind="Internal").ap()

        # ============ phase C: attention ===========================
        with tc.tile_pool(name="st", bufs=1, space="PSUM") as P_ST, \
             tc.tile_pool(name="ot", bufs=1, space="PSUM") as P_OT:
            for g in range(3):                  # head quads
                otb = P_OT.tile([65, 4, QPC], f32, tag="ot")
                prev = None

                def pv_flush(g=g, otb=otb):
                    nonlocal prev
                    if prev is None:
                        return
                    pk, pp = prev
                    pq0 = 16 * pk
                    for a in range(4):
                        nc.tensor.matmul(
                            otb[:, a, pq0:QPC], VSB[:, pk, 4 * g + a, :],
                            pp[:, a, :], start=(pk == 0),
                            stop=(pk == NKV - 1))
                    prev = None

                for k in range(NKV):            # kv blocks
                    q0 = 16 * k
                    n = QPC - q0
                    halves = []
                    for hb in range(2):         # half: heads {2hb, 2hb+1}
                        stb = P_ST.tile([128, 2, 512], f32, tag=f"st{hb}")
                        for eo in range(2):
                            for aa in range(2):
                                a = 2 * hb + aa
                                tp = (96, 0) if a == 3 else None
                                nc.tensor.matmul(
                                    stb[:, aa, 0:n],
                                    KT[32 * a:32 * (a + 1), eo, g,
                                       128 * k:128 * (k + 1)],
                                    QT[32 * a:32 * (a + 1), eo, g, q0:QPC],
                                    start=(eo == 0), stop=(eo == 1),
                                    tile_position=tp)
                        halves.append(stb)
                    pv_flush()
                    p = P_P.tile([128, 4, n], bf, tag="p")
                    for hb in range(2):
                        nc.scalar.activation(
                            p[:, 2 * hb:2 * hb + 2, :], halves[hb][:, :, 0:n],
                            mybir.ActivationFunctionType.Exp, scale=SCALE)
                    nc.vector.tensor_mul(
                        p[:, :, 0:16], p[:, :, 0:16],
                        msk[:, None, :].broadcast_to((128, 4, 16)))
                    prev = (k, p)
                pv_flush()

                # normalize: reciprocals + unnormalized copies on DVE; the
                # broadcast DMA bounce + in-place multiply overlap the next
                # quad's attention.
                r1 = P_N.tile([1, 4 * QPC], bf, tag="r1")
                with nc.allow_low_precision(reason="bf16 softmax denom"):
                    for a in range(4):
                        nc.vector.reciprocal(r1[:, QPC * a:QPC * (a + 1)],
                                             otb[64:65, a, :])
                for half in range(2):
                    nc.vector.tensor_copy(
                        OTSB[64 * half:64 * half + 64, 2 * g:2 * g + 2, :],
                        otb[0:64, half::2, :])
                nc.sync.dma_start(nrmB_d[g:g + 1, :], r1[:])
                rb = P_N.tile([128, 4 * QPC], bf, tag="rb")
                nc.sync.dma_start(rb[:],
                                  nrmB_d[g:g + 1, :].to_broadcast((128, 4 * QPC)))
                rb4 = rb[:].rearrange("p (a q) -> p a q", a=4)
                for a in range(4):
                    h = 4 * g + a
                    pb = 64 * (h % 2)
                    dst = OTSB[pb:pb + 64, h // 2, :]
                    nc.vector.tensor_mul(dst, dst, rb4[pb:pb + 64, a, :])

        # ============ phase D: output projection ===================
        with tc.tile_pool(name="pd", bufs=1, space="PSUM") as P_PD:
            for j in range(4):                  # q sub-tiles of 128
                pss = []
                for nh in range(2):
                    ps = P_PD.tile([128, 384], f32, tag=f"ops{nh}")
                    for s in range(NDS):
                        nc.tensor.matmul(
                            ps[:], OTSB[:, s, 128 * j:128 * (j + 1)],
                            wo_sb[:, s, 384 * nh:384 * (nh + 1)],
                            start=(s == 0), stop=(s == NDS - 1))
                    pss.append(ps)
                ob = P_O.tile([128, D], f32, tag="ob")
                nc.scalar.copy(ob[:, 0:384], pss[0][:])
                nc.scalar.copy(ob[:, 384:768], pss[1][:])
                nc.sync.dma_start(out_d[128 * j:128 * (j + 1), :], ob[:])

    nc.compile()
    return nc


def _prep_inputs(x, wq, wk, wv, wo, token_positions):
    import ml_dtypes
    bf16 = ml_dtypes.bfloat16

    x2 = np.ascontiguousarray(x[0], dtype=np.float32)          # [S, D]
    xT = np.ascontiguousarray(x2.T).astype(bf16)               # [D, S]
    perm = _head_perm()
    wqT = np.ascontiguousarray(wq[perm, :].T).astype(bf16)
    wkT = np.ascontiguousarray(wk[perm, :].T).astype(bf16)
    wvT = np.ascontiguousarray(wv.T).astype(bf16)
    woT = np.ascontiguousarray(wo.T).astype(bf16)

    pos = np.asarray(token_positions[0], dtype=np.int64)       # [S]
    kk = np.arange(HD2, dtype=np.float32)
    inv = (10000.0 ** (-2.0 * kk / HD)).astype(np.float32)
    ang = pos[:, None].astype(np.float32) * inv[None, :]       # [S, 32]
    cosf = np.cos(ang, dtype=np.float32)
    sinf = np.sin(ang, dtype=np.float32)
    cosK = np.ascontiguousarray(np.tile(cosf[:SLOC].T, (4, 1))).astype(bf16)
    sinK = np.ascontiguousarray(np.tile(sinf[:SLOC].T, (4, 1))).astype(bf16)

    xTloc = np.ascontiguousarray(xT[:, :SLOC])

    per_core = []
    for c in range(NCORES):
        lo = SLOC + W * c
        xg = np.ascontiguousarray(xT[:, lo:lo + W])
        cosG = np.ascontiguousarray(
            np.tile(cosf[lo:lo + W].T, (4, 1))).astype(bf16)
        sinG = np.ascontiguousarray(
            np.tile(sinf[lo:lo + W].T, (4, 1))).astype(bf16)
        xq = np.ascontiguousarray(xT[:, c::NCORES])            # [D, 512]
        cosQ = np.ascontiguousarray(
            np.tile(cosf[c::NCORES].T, (4, 1))).astype(bf16)
        sinQ = np.ascontiguousarray(
            np.tile(sinf[c::NCORES].T, (4, 1))).astype(bf16)
        kl = np.arange(128)[:, None]
        jj = np.arange(16)[None, :]
        mask = (kl <= 8 * jj + c).astype(np.float32).astype(bf16)
        per_core.append({
            "xT": xTloc, "xg": xg, "xq": xq,
            "wqT": wqT, "wkT": wkT, "wvT": wvT, "woT": woT,
            "cosK": cosK, "sinK": sinK, "cosG": cosG, "sinG": sinG,
            "cosQ": cosQ, "sinQ": sinQ, "mask": mask,
        })
    return per_core


def kernel(x, wq, wk, wv, wo, token_positions):
    global last_exec_time_ns, last_results
    import os
    from concourse import bass_utils

    key = "v2"
    if key not in _CACHE:
        _CACHE[key] = _build_program()
    nc = _CACHE[key]

    in_maps = _prep_inputs(np.asarray(x), np.asarray(wq), np.asarray(wk),
                           np.asarray(wv), np.asarray(wo),
                           np.asarray(token_positions))

    kw = {}
    if os.environ.get("BASS_KERNEL_TRACE", "0") == "1":
        kw = dict(trace=True,
                  trace_cores=[int(t) for t in os.environ.get(
                      "BASS_KERNEL_TRACE_CORES", "0").split(",")])
        td = os.environ.get("BASS_KERNEL_TMPDIR")
        if td:
            kw["tmpdir"] = td
    res = bass_utils.run_bass_kernel_spmd(nc, in_maps,
                                          core_ids=list(range(NCORES)), **kw)
    last_exec_time_ns = res.exec_time_ns
    last_results = res

    out = np.empty((S, D), dtype=np.float32)
    for c in range(NCORES):
        out[c::NCORES, :] = res.results[c]["out"]
    return out[None, :, :]
